# revision 1
# baseline (speedup 1.0000x reference)
"""Trainium2 Bass kernel for Mistral-style sliding-window GQA attention.

Problem: B=2, T=2048, C=2048, 32 q heads / 8 kv heads, head_dim=64,
sliding causal window 1024, RoPE, fp32.

Sharding (sequence-parallel, no cross-core communication):
  core c in 0..7 handles batch b=c//4 and contiguous 512-row chunk k=c%4.
  Each core computes q for its 512 rows, k/v for its rows plus a 1024-row
  halo (zero-padded before t=0), full attention for its rows over all 32
  heads, and the output projection for its rows.  Host gathers by
  concatenation only.

Device program details:
  - float32r (single-pass fp32 PE mode, ~1.5e-4 matmul error) for all
    matmul operands; PSUM accumulation stays fp32.
  - x is transposed on host; RoPE cos/sin tables and masks are host inputs.
  - scores are built in S^T = [key, query] layout, with the 4 query heads of
    each kv group packed side-by-side in the moving operand (N=512 matmuls).
  - PV uses V as the stationary operand: out^T = V_ext^T @ P^T accumulates
    [65, 4x128] per (group, q-tile); row 64 (from the validity column of
    V_ext) is the softmax denominator, and rows 0..63 are already in the
    aT=[d, t] layout the output projection needs - no transposes anywhere.
  - no max-subtraction in softmax: inputs are N(0,1)-scaled so |scores/8|
    stays ~15; exp is safe in fp32.
  - Q projection runs as four 8-head quarter-sweeps interleaved with
    attention over the heads already produced, so ACT-bound softmax overlaps
    PE-bound projection.
  - invalid (zero-padded halo) keys get exp(0)=1 scores but contribute zero
    to both PV numerator and the validity-column denominator.
"""

import os
import numpy as np

import concourse.bass as bass
import concourse.mybir as mybir
import concourse.tile as tile
from concourse import bacc
from concourse.bass_utils import run_bass_kernel_spmd

B, T, C = 2, 2048, 2048
NH, NKV, D = 32, 8, 64
REP = NH // NKV
WIN = 1024
CH = 512          # q rows per core
KVR = CH + WIN    # kv rows per core (with halo)
NCORE = 8
DT = mybir.dt.float32
F32R = mybir.dt.float32r
SCALE = 1.0 / np.sqrt(np.float32(D))
ROPE_BASE = 10000.0

FD = T // 128     # 16 contraction tiles of the model dim
NQT = CH // 128   # 4 q tiles per chunk
NKB = KVR // 128  # 12 kv blocks per core
NWB = 9           # kv blocks in the window of one q tile
VW = 65           # v_ext width per kv block (64 dims + validity column)
VP = NKB * VW     # per-head v_ext pitch (780)


def _rope_write(nc, pool, out_ap, ps, cosw, ssinw, n, swap_engine=None):
    """out = ps*cos + rot_half(ps)*sin on a [128, n] 2-head-packed tile.

    ssinw rows carry the rotate-half signs (rows 0-31/64-95 negated) and any
    folded scale; cosw carries the same scale.  out_ap is either one [128, n]
    AP or a list of two ([64, n] AP) halves receiving rows 0:64 / 64:128.

    If swap_engine is given (an idle PSUM-capable engine, e.g. nc.scalar),
    the rotate-half shuffle is materialized there with 4 quarter copies and
    the vector engine does only 3 full-width ops; otherwise the vector
    engine does 4 quarter multiplies + 2 full ops.
    """
    if swap_engine is not None:
        sw = pool.tile([128, n], DT, tag="rope_sw", name="rope_sw")
        swap_engine.copy(sw[0:32, :], ps[32:64, :])
        swap_engine.copy(sw[32:64, :], ps[0:32, :])
        swap_engine.copy(sw[64:96, :], ps[96:128, :])
        swap_engine.copy(sw[96:128, :], ps[64:96, :])
        t2 = pool.tile([128, n], DT, tag="rope_t2", name="rope_t2")
        nc.vector.tensor_mul(t2[:], sw[:], ssinw[:])
    else:
        t2 = pool.tile([128, n], DT, tag="rope_t2", name="rope_t2")
        nc.vector.tensor_mul(t2[0:32, :], ps[32:64, :], ssinw[0:32, :])
        nc.vector.tensor_mul(t2[32:64, :], ps[0:32, :], ssinw[32:64, :])
        nc.vector.tensor_mul(t2[64:96, :], ps[96:128, :], ssinw[64:96, :])
        nc.vector.tensor_mul(t2[96:128, :], ps[64:96, :], ssinw[96:128, :])
    t1 = pool.tile([128, n], DT, tag="rope_t1", name="rope_t1")
    nc.vector.tensor_mul(t1[:], ps[:], cosw[:])
    if isinstance(out_ap, list):
        for i, half in enumerate(out_ap):
            nc.gpsimd.tensor_add(half, t1[64 * i:64 * (i + 1), :],
                                 t2[64 * i:64 * (i + 1), :])
    else:
        nc.gpsimd.tensor_add(out_ap, t1[:], t2[:])


def build_program():
    nc = bacc.Bacc("TRN2", target_bir_lowering=False, debug=False,
                   num_devices=NCORE)

    xkv_d = nc.dram_tensor("xkv", [C, KVR], F32R, kind="ExternalInput")
    wq_d = nc.dram_tensor("wq", [C, NH * D], F32R, kind="ExternalInput")
    wk_d = nc.dram_tensor("wk", [C, NKV * D], F32R, kind="ExternalInput")
    wv_d = nc.dram_tensor("wv", [C, NKV * D], F32R, kind="ExternalInput")
    wo_d = nc.dram_tensor("wo", [NH * D, C], F32R, kind="ExternalInput")
    rqc_d = nc.dram_tensor("rope_q_cos", [128, CH], DT, kind="ExternalInput")
    rqs_d = nc.dram_tensor("rope_q_sin", [128, CH], DT, kind="ExternalInput")
    rkc_d = nc.dram_tensor("rope_k_cos", [128, KVR], DT, kind="ExternalInput")
    rks_d = nc.dram_tensor("rope_k_sin", [128, KVR], DT, kind="ExternalInput")
    kvv_d = nc.dram_tensor("kvvalid", [128, NKB], F32R, kind="ExternalInput")
    mw_d = nc.dram_tensor("mask_win8", [128, 512], F32R, kind="ExternalInput")
    mc_d = nc.dram_tensor("mask_causal8", [128, 512], F32R,
                          kind="ExternalInput")
    out_d = nc.dram_tensor("out", [CH, C], DT, kind="ExternalOutput")

    with tile.TileContext(nc) as tc:
        with (
            tc.tile_pool(name="const", bufs=1) as cpool,
            tc.tile_pool(name="qT", bufs=1) as qT_pool,
            tc.tile_pool(name="kT", bufs=1) as kT_pool,
            tc.tile_pool(name="vext", bufs=1) as v_pool,
        ):
            # ---- constants (small, persistent) ----
            mask_win = cpool.tile([128, 512], F32R, tag="mw", name="mask_win")
            nc.gpsimd.dma_start(mask_win[:], mw_d[:, :])
            mask_causal = cpool.tile([128, 512], F32R, tag="mc",
                                     name="mask_causal")
            nc.gpsimd.dma_start(mask_causal[:], mc_d[:, :])
            kvv = cpool.tile([128, NKB], F32R, tag="kvv", name="kvv")
            nc.gpsimd.dma_start(kvv[:], kvv_d[:, :])

            # qT: [d, t] grouped by kv head.  Tile j rows 0:64 = group 2j
            # (its 4 heads side by side, 512 cols each), rows 64:128 =
            # group 2j+1, so QK lhsT and rhs share a base partition.
            qT = [qT_pool.tile([128, REP * CH], F32R, tag=f"qT{i}",
                               name=f"qT{i}") for i in range(NKV // 2)]
            # kT: [d, t] packed 2 kv heads per tile.
            kT = [kT_pool.tile([128, KVR], F32R, tag=f"kT{i}", name=f"kT{i}")
                  for i in range(NKV // 2)]
            # vext: one tile, head kvh at pitch VP; per block 64 dims+validity
            vext = v_pool.tile([128, NKV * VP], F32R, tag="vext", name="vext")

            # ================= KV projection =================
            with (
                tc.tile_pool(name="rk_tab", bufs=1) as rk_pool,
                tc.tile_pool(name="wk_res", bufs=1) as wk_pool,
                tc.tile_pool(name="wv_res", bufs=1) as wv_pool,
                tc.tile_pool(name="xkv_s", bufs=6) as xkv_pool,
                tc.tile_pool(name="rope_tmp", bufs=3) as rtmp,
                tc.tile_pool(name="ps_kv", bufs=1, space="PSUM") as ps_kv,
            ):
                rkc = rk_pool.tile([128, KVR], DT, tag="rkc", name="rkc")
                nc.gpsimd.dma_start(rkc[:], rkc_d[:, :])
                rks = rk_pool.tile([128, KVR], DT, tag="rks", name="rks")
                nc.gpsimd.dma_start(rks[:], rks_d[:, :])
                wkt = {}
                wvt = {}
                for ci in range(FD):
                    wkt[ci] = wk_pool.tile([128, NKV * D], F32R,
                                           tag=f"wk{ci}", name=f"wk{ci}")
                    nc.scalar.dma_start(
                        wkt[ci][:], wk_d[128 * ci:128 * (ci + 1), :])
                    wvt[ci] = wv_pool.tile([128, NKV * D], F32R,
                                           tag=f"wv{ci}", name=f"wv{ci}")
                    nc.scalar.dma_start(
                        wvt[ci][:], wv_d[128 * ci:128 * (ci + 1), :])

                NTH = 3          # x-column thirds
                QW = KVR // NTH  # 512 columns per third
                for qu in range(NTH):
                    qs = QW * qu
                    kps = [ps_kv.tile([128, QW], DT, tag=f"kps{m}",
                                      name=f"kps{m}")
                           for m in range(NKV // 2)]
                    vps = [ps_kv.tile([128, NKV * D], DT, tag=f"vps{st}",
                                      name=f"vps{st}")
                           for st in range(QW // 128)]
                    for ci in range(FD):
                        xt = xkv_pool.tile([128, QW], F32R, tag="xkv",
                                           name="xkv")
                        nc.sync.dma_start(
                            xt[:], xkv_d[128 * ci:128 * (ci + 1), qs:qs + QW])
                        for m in range(NKV // 2):
                            nc.tensor.matmul(
                                kps[m][:], wkt[ci][:, 128 * m:128 * (m + 1)],
                                xt[:], start=(ci == 0), stop=(ci == FD - 1))
                        for st in range(QW // 128):
                            nc.tensor.matmul(
                                vps[st][:], xt[:, 128 * st:128 * (st + 1)],
                                wvt[ci][:], start=(ci == 0),
                                stop=(ci == FD - 1))
                    for m in range(NKV // 2):
                        _rope_write(nc, rtmp, kT[m][:, qs:qs + QW],
                                    kps[m][:], rkc[:, qs:qs + QW],
                                    rks[:, qs:qs + QW], QW,
                                    swap_engine=nc.scalar)
                    for st in range(QW // 128):
                        tl = (QW // 128) * qu + st   # kv block 0..11
                        # v data for all 8 heads in one strided copy
                        nc.scalar.copy(
                            vext[:].rearrange("p (h b w) -> p h b w",
                                              h=NKV, b=NKB)[:, :, tl, 0:D],
                            vps[st][:].rearrange("p (h d) -> p h d", h=NKV))
                    # validity columns for this third's blocks, all heads
                    t0 = (QW // 128) * qu
                    nc.scalar.copy(
                        vext[:].rearrange("p (h b w) -> p h b w",
                                          h=NKV, b=NKB)[
                                              :, :, t0:t0 + QW // 128,
                                              D:D + 1],
                        kvv[:, t0:t0 + QW // 128].rearrange(
                            "p (o b) -> p o b", o=1).to_broadcast(
                                (128, NKV, QW // 128)))

            # ====== interleaved Q projection + attention ladder ======
            with (
                tc.tile_pool(name="aT", bufs=1) as aT_pool,
                tc.tile_pool(name="rq_tab", bufs=1) as rq_pool,
                tc.tile_pool(name="wq_s", bufs=4) as wq_pool,
                tc.tile_pool(name="xq_s", bufs=8) as xq_pool,
                tc.tile_pool(name="rope_tmp_q", bufs=3) as rtmpq,
                tc.tile_pool(name="pt", bufs=6) as pt_pool,
                tc.tile_pool(name="att_small", bufs=2) as sm_pool,
                tc.tile_pool(name="ps_att", bufs=1, space="PSUM") as ps_att,
            ):
                aT = [aT_pool.tile([128, CH], F32R, tag=f"aT{i}",
                                   name=f"aT{i}") for i in range(NH // 2)]
                rqc = rq_pool.tile([128, CH], DT, tag="rqc", name="rqc")
                nc.gpsimd.dma_start(rqc[:], rqc_d[:, :])
                rqs = rq_pool.tile([128, CH], DT, tag="rqs", name="rqs")
                nc.gpsimd.dma_start(rqs[:], rqs_d[:, :])

                def q_quarter(sweep, swap_engine=None):
                    # projects heads 8*sweep .. 8*sweep+7 (psum m-tiles
                    # 4*sweep .. 4*sweep+3)
                    qps = [ps_q.tile([128, CH], DT, tag=f"qps{m4}",
                                     name=f"qps{m4}") for m4 in range(4)]
                    for ci in range(FD):
                        xt = xq_pool.tile([128, CH], F32R, tag="xq", name="xq")
                        nc.sync.dma_start(
                            xt[:], xkv_d[128 * ci:128 * (ci + 1),
                                         WIN:WIN + CH])
                        wt = wq_pool.tile([128, 512], F32R, tag="wq",
                                          name="wq")
                        nc.sync.dma_start(
                            wt[:], wq_d[128 * ci:128 * (ci + 1),
                                        512 * sweep:512 * (sweep + 1)])
                        for m4 in range(4):
                            nc.tensor.matmul(qps[m4][:],
                                             wt[:, 128 * m4:128 * (m4 + 1)],
                                             xt[:], start=(ci == 0),
                                             stop=(ci == FD - 1))
                    for m4 in range(4):
                        m = 4 * sweep + m4
                        boff = 64 * ((m // 2) % 2)
                        tau = m // 4
                        c0 = 512 * (2 * (m % 2))
                        _rope_write(nc, rtmpq,
                                    [qT[tau][boff:boff + 64, c0:c0 + 512],
                                     qT[tau][boff:boff + 64,
                                             c0 + 512:c0 + 1024]],
                                    qps[m4][:], rqc[:], rqs[:], CH,
                                    swap_engine=swap_engine)

                def attention_pair(gp, sb=2, ob=2, sfx=""):
                    for g in (2 * gp, 2 * gp + 1):
                        kTt, koff = kT[g // 2], 64 * (g % 2)
                        qTg = qT[g // 2]
                        for qt in range(NQT):
                            qv = qTg[koff:koff + 64, :].rearrange(
                                "p (r t) -> p r t", r=REP)[
                                    :, :, 128 * qt:128 * (qt + 1)]
                            OT = ps_att.tile([65, REP * 128], DT,
                                             tag="OT" + sfx, name="OT",
                                             bufs=ob)
                            for lk in range(NWB):
                                kb = qt + lk
                                ST = ps_att.tile([128, REP * 128], DT,
                                                 tag="ST" + sfx, name="ST",
                                                 bufs=sb)
                                nc.tensor.matmul(
                                    ST.rearrange("p (r t) -> p r t", r=REP),
                                    kTt[koff:koff + 64,
                                        128 * kb:128 * (kb + 1)],
                                    qv, start=True, stop=True)
                                PT = pt_pool.tile([128, REP * 128], F32R,
                                                  tag="PT", name="PT", bufs=5)
                                nc.scalar.activation(
                                    PT[:], ST[:],
                                    mybir.ActivationFunctionType.Exp)
                                if lk == 0:
                                    nc.vector.tensor_mul(PT[:], PT[:],
                                                         mask_win[:])
                                elif lk == NWB - 1:
                                    nc.vector.tensor_mul(PT[:], PT[:],
                                                         mask_causal[:])
                                nc.tensor.matmul(
                                    OT[:],
                                    vext[:, VP * g + VW * kb:
                                         VP * g + VW * (kb + 1)],
                                    PT[:], start=(lk == 0),
                                    stop=(lk == NWB - 1))
                            rcp = sm_pool.tile([1, REP * 128], DT,
                                               tag="rcp", name="rcp")
                            nc.vector.reciprocal(rcp[:], OT[64:65, :])
                            rcpb = sm_pool.tile([64, REP * 128], DT,
                                                tag="rcpb", name="rcpb")
                            nc.gpsimd.partition_broadcast(rcpb[:], rcp[:])
                            for r in range(REP):
                                h = REP * g + r
                                nc.vector.tensor_mul(
                                    aT[h // 2][64 * (h % 2):
                                               64 * (h % 2) + 64,
                                               128 * qt:128 * (qt + 1)],
                                    OT[0:64, 128 * r:128 * (r + 1)],
                                    rcpb[:, 128 * r:128 * (r + 1)])

                with tc.tile_pool(name="ps_q", bufs=1, space="PSUM") as ps_q:
                    for sweep in range(4):
                        if sweep >= 1:
                            attention_pair(sweep - 1)
                        q_quarter(sweep,
                                  swap_engine=nc.scalar if sweep == 0
                                  else None)
                attention_pair(3)

                # ================= output projection =================
                with (
                    tc.tile_pool(name="wo_s", bufs=10) as wo_pool,
                    tc.tile_pool(name="ostage", bufs=3) as ostage,
                    tc.tile_pool(name="ps_o", bufs=1, space="PSUM") as ps_o,
                ):
                    for oc in range(4):
                        ops = [ps_o.tile([128, 512], DT, tag=f"ops{tt}",
                                         name=f"ops{tt}")
                               for tt in range(NQT)]
                        for k in range(FD):
                            wot = wo_pool.tile([128, 512], F32R, tag="wo",
                                               name="wo")
                            eng = nc.scalar if k % 2 else nc.sync
                            eng.dma_start(
                                wot[:], wo_d[128 * k:128 * (k + 1),
                                             512 * oc:512 * (oc + 1)])
                            for tt in range(NQT):
                                nc.tensor.matmul(
                                    ops[tt][:],
                                    aT[k][:, 128 * tt:128 * (tt + 1)],
                                    wot[:], start=(k == 0),
                                    stop=(k == FD - 1))
                        for tt in range(NQT):
                            st = ostage.tile([128, 512], DT, tag="stage",
                                             name="stage")
                            nc.vector.tensor_copy(st[:], ops[tt][:])
                            nc.gpsimd.dma_start(
                                out_d[128 * tt:128 * (tt + 1),
                                      512 * oc:512 * (oc + 1)], st[:])

    nc.compile()
    return nc


def _rope_tables(t_idx, scale):
    """cos/sin tables in [d, t] layout, 2-head packed to 128 partitions.

    Rows 0-63 and 64-127 identical; sin rows 0-31 (and 64-95) carry the
    rotate-half minus sign."""
    inv_freq = 1.0 / (ROPE_BASE ** (np.arange(0, D, 2, dtype=np.float64) / D))
    ang = t_idx[None, :] * inv_freq[:, None]          # [32, n]
    cos1 = np.cos(ang)
    sin1 = np.sin(ang)
    cos64 = np.concatenate([cos1, cos1], 0) * scale   # [64, n]
    sin64 = np.concatenate([-sin1, sin1], 0) * scale  # [64, n] signed
    return (np.tile(cos64, (2, 1)).astype(np.float32),
            np.tile(sin64, (2, 1)).astype(np.float32))


def make_in_maps(x, Wq, Wk, Wv, Wo):
    x = np.asarray(x, np.float32)
    ins = []
    i = np.arange(128)
    masks = {
        "mask_win8": np.tile((i[:, None] > i[None, :]).astype(np.float32),
                             (1, REP)),
        "mask_causal8": np.tile((i[:, None] <= i[None, :]).astype(np.float32),
                                (1, REP)),
    }
    for c in range(NCORE):
        b, ch = divmod(c, 4)
        r0 = CH * ch
        kv0 = r0 - WIN
        xT = np.ascontiguousarray(x[b].T)             # [C, T]
        xkv = np.zeros((C, KVR), np.float32)
        pad = max(0, -kv0)
        xkv[:, pad:] = xT[:, kv0 + pad:r0 + CH]
        qc, qs = _rope_tables(np.arange(r0, r0 + CH, dtype=np.float64), SCALE)
        kc, ks = _rope_tables(np.arange(kv0, r0 + CH, dtype=np.float64), 1.0)
        kvvalid = np.zeros((128, NKB), np.float32)
        for lk in range(NKB):
            kvvalid[:, lk] = (kv0 + 128 * lk + i >= 0).astype(np.float32)
        ins.append({
            "xkv": xkv,
            "wq": np.ascontiguousarray(Wq, np.float32),
            "wk": np.ascontiguousarray(Wk, np.float32),
            "wv": np.ascontiguousarray(Wv, np.float32),
            "wo": np.ascontiguousarray(Wo, np.float32),
            "rope_q_cos": qc, "rope_q_sin": qs,
            "rope_k_cos": kc, "rope_k_sin": ks,
            "kvvalid": kvvalid,
            **masks,
        })
    return ins


_PROG_CACHE = {}


def get_program():
    if "nc" not in _PROG_CACHE:
        _PROG_CACHE["nc"] = build_program()
    return _PROG_CACHE["nc"]


def kernel(x, Wq, Wk, Wv, Wo):
    nc = get_program()
    ins = make_in_maps(x, Wq, Wk, Wv, Wo)
    res = run_bass_kernel_spmd(nc, ins, list(range(NCORE)))
    out = np.empty((B, T, C), np.float32)
    for c in range(NCORE):
        b, ch = divmod(c, 4)
        out[b, CH * ch:CH * (ch + 1), :] = res.results[c]["out"]
    return out



# revision 18
# speedup vs baseline: 1.0474x; 1.0474x over previous
"""Trainium2 Bass kernel for Mistral-style sliding-window GQA attention.

Problem: B=2, T=2048, C=2048, 32 q heads / 8 kv heads, head_dim=64,
sliding causal window 1024, RoPE, fp32.

Sharding (sequence-parallel, no cross-core communication):
  core c in 0..7 handles batch b=c//4 and contiguous 512-row chunk k=c%4.
  Each core computes q for its 512 rows, k/v for its rows plus a 1024-row
  halo (zero-padded before t=0), full attention for its rows over all 32
  heads, and the output projection for its rows.  Host gathers by
  concatenation only.

Device program (v2):
  - All DRAM inputs are fp16 (host-converted); matmul operands are fp16
    except the softmax path (PT / vext in bf16 for exp range safety);
    PSUM accumulation stays fp32.
  - All bulk weight/x DMAs ride the SP (sync) queue only, so HWDGE is
    uncontended and the ACT/DVE sequencers never stall behind DMA issue.
  - Q projection reuses the third-2 x tiles of the KV phase (the q token
    range is rows 1024:1536 of the kv range) - zero x re-loads.
  - RoPE: one ACT stage-copy (PSUM f32 -> SBUF fp16), then 6 DVE ops in
    fp16 (4 quarter muls for rotate-half * sin, full cos mul, full add).
  - Attention pairs the two kv groups of a ladder step into one
    [128, 1024] PSUM score tile so each Exp activation covers 2 groups.
  - scores are built in S^T = [key, query] layout with the 4 query heads
    packed side-by-side; PV uses V_ext as stationary so out^T lands in
    [d, t] with the softmax denominator in row 64 - no transposes.
  - no max-subtraction in softmax (scores/8 bounded ~15, exp safe in f32).
  - output projection accumulates in PSUM and DMAs PSUM->DRAM directly.
"""

import numpy as np
import ml_dtypes

import concourse.bass as bass
import concourse.mybir as mybir
import concourse.tile as tile
from concourse import bacc
from concourse.bass_utils import run_bass_kernel_spmd

B, T, C = 2, 2048, 2048
NH, NKV, D = 32, 8, 64
REP = NH // NKV
WIN = 1024
CH = 512          # q rows per core
KVR = CH + WIN    # kv rows per core (with halo)
NCORE = 8
DT = mybir.dt.float32
F16 = mybir.dt.float16
BF16 = mybir.dt.bfloat16
SCALE = 1.0 / np.sqrt(np.float32(D))
ROPE_BASE = 10000.0

FD = T // 128     # 16 contraction tiles of the model dim
NQT = CH // 128   # 4 q tiles per chunk
NKB = KVR // 128  # 12 kv blocks per core
NWB = 9           # kv blocks in the window of one q tile
VW = 65           # v_ext width per kv block (64 dims + validity column)
VP = NKB * VW     # per-head v_ext pitch (780)

# Head dims are PAIR-INTERLEAVED on host (new dim 2j = old j, 2j+1 = old
# j+32), so RoPE rotate-half is an adjacent-partition swap - expressible as
# a stream_shuffle (which permutes within 32-partition quadrants only).
_PAIR_SHUF = [i ^ 1 for i in range(32)]


def _rope_write(nc, pool, out_ap, ps, cosw, ssinw, n):
    """out = ps*cos + rot_half(ps)*sin on a [128, n] 2-head-packed tile.

    ps is a PSUM f32 tile; one ACT copy stages it to fp16 SBUF, then all
    multiplies/adds run on DVE in fp16.  ssinw rows carry the rotate-half
    signs (rows 0-31/64-95 negated) and any folded scale; cosw carries the
    same scale.  out_ap receives the fp16 result.
    """
    rs = pool.tile([128, n], F16, tag="rope_rs", name="rope_rs")
    nc.scalar.copy(rs[:], ps[:])
    # rotate-half = adjacent-partition swap in the pair-interleaved layout
    sw = pool.tile([128, n], F16, tag="rope_sw", name="rope_sw")
    nc.vector.stream_shuffle(sw[:], rs[:], _PAIR_SHUF)
    t2 = pool.tile([128, n], F16, tag="rope_t2", name="rope_t2")
    nc.vector.tensor_mul(t2[:], sw[:], ssinw[:])
    t1 = pool.tile([128, n], F16, tag="rope_t1", name="rope_t1")
    nc.vector.tensor_mul(t1[:], rs[:], cosw[:])
    nc.vector.tensor_add(out_ap, t1[:], t2[:])


def build_program():
    nc = bacc.Bacc("TRN2", target_bir_lowering=False, debug=False,
                   num_devices=NCORE)

    xkv_d = nc.dram_tensor("xkv", [C, KVR], F16, kind="ExternalInput")
    wq_d = nc.dram_tensor("wq", [C, NH * D], F16, kind="ExternalInput")
    wk_d = nc.dram_tensor("wk", [C, NKV * D], F16, kind="ExternalInput")
    wv_d = nc.dram_tensor("wv", [C, NKV * D], F16, kind="ExternalInput")
    wo_d = nc.dram_tensor("wo", [NH * D, C], F16, kind="ExternalInput")
    rqc_d = nc.dram_tensor("rope_q_cos", [128, CH], F16, kind="ExternalInput")
    rqs_d = nc.dram_tensor("rope_q_sin", [128, CH], F16, kind="ExternalInput")
    rkc_d = nc.dram_tensor("rope_k_cos", [128, KVR], F16, kind="ExternalInput")
    rks_d = nc.dram_tensor("rope_k_sin", [128, KVR], F16, kind="ExternalInput")
    kvv_d = nc.dram_tensor("kvvalid", [128, NKB], BF16, kind="ExternalInput")
    mw_d = nc.dram_tensor("mask_win8", [128, 1024], BF16, kind="ExternalInput")
    mc_d = nc.dram_tensor("mask_causal8", [128, 1024], BF16,
                          kind="ExternalInput")
    out_d = nc.dram_tensor("out", [CH, C], DT, kind="ExternalOutput")

    with tile.TileContext(nc) as tc:
        with (
            tc.tile_pool(name="const", bufs=1) as cpool,
            tc.tile_pool(name="qT", bufs=1) as qT_pool,
            tc.tile_pool(name="kT", bufs=1) as kT_pool,
            tc.tile_pool(name="vext", bufs=1) as v_pool,
            tc.tile_pool(name="x2", bufs=1) as x2_pool,
            tc.tile_pool(name="wq_s", bufs=1) as wq_pool,
        ):
            # ---- constants (small, persistent; SWDGE queue) ----
            mask_win = cpool.tile([128, 1024], BF16, tag="mw", name="mask_win")
            nc.gpsimd.dma_start(mask_win[:], mw_d[:, :])
            mask_causal = cpool.tile([128, 1024], BF16, tag="mc",
                                     name="mask_causal")
            nc.gpsimd.dma_start(mask_causal[:], mc_d[:, :])
            kvv = cpool.tile([128, NKB], BF16, tag="kvv", name="kvv")
            nc.gpsimd.dma_start(kvv[:], kvv_d[:, :])
            rkc = cpool.tile([128, KVR], F16, tag="rkc", name="rkc")
            nc.gpsimd.dma_start(rkc[:], rkc_d[:, :])
            rks = cpool.tile([128, KVR], F16, tag="rks", name="rks")
            nc.gpsimd.dma_start(rks[:], rks_d[:, :])
            rqc = cpool.tile([128, CH], F16, tag="rqc", name="rqc")
            nc.gpsimd.dma_start(rqc[:], rqc_d[:, :])
            rqs = cpool.tile([128, CH], F16, tag="rqs", name="rqs")
            nc.gpsimd.dma_start(rqs[:], rqs_d[:, :])

            # qT: [d, t] grouped by kv head.  Tile j rows 0:64 = group 2j
            # (its 4 heads side by side, 512 cols each), rows 64:128 =
            # group 2j+1, so QK lhsT and rhs share a base partition.
            qT = [qT_pool.tile([128, REP * CH], F16, tag=f"qT{i}",
                               name=f"qT{i}") for i in range(NKV // 2)]
            # kT: [d, t] packed 2 kv heads per tile.
            kT = [kT_pool.tile([128, KVR], F16, tag=f"kT{i}", name=f"kT{i}")
                  for i in range(NKV // 2)]
            # vext: one tile, head kvh at pitch VP; per block 64 dims+validity
            vext = v_pool.tile([128, NKV * VP], BF16, tag="vext", name="vext")
            # third-2 x tiles stay resident: they double as the q-range x.
            x2t = [x2_pool.tile([128, CH], F16, tag=f"x2_{ci}",
                                name=f"x2_{ci}") for ci in range(FD)]
            # wq tiles: rotating pool, prefetched 2 sweeps ahead (SP queue)
            wq_tiles = {}

            def prefetch_wq(s):
                if s > 3 or s in wq_tiles:
                    return
                tl = []
                for ci in range(FD):
                    t = wq_pool.tile([128, 512], F16, tag="wq", name="wq",
                                     bufs=32)
                    nc.sync.dma_start(
                        t[:], wq_d[128 * ci:128 * (ci + 1),
                                   512 * s:512 * (s + 1)])
                    tl.append(t)
                wq_tiles[s] = tl

            # ================= KV projection =================
            with (
                tc.tile_pool(name="wk_res", bufs=1) as wk_pool,
                tc.tile_pool(name="wv_res", bufs=1) as wv_pool,
                tc.tile_pool(name="xkv_s", bufs=6) as xkv_pool,
                tc.tile_pool(name="rope_tmp", bufs=3) as rtmp,
                tc.tile_pool(name="ps_kv", bufs=1, space="PSUM") as ps_kv,
            ):
                wkt = {}
                wvt = {}
                for ci in range(FD):
                    wkt[ci] = wk_pool.tile([128, NKV * D], F16,
                                           tag=f"wk{ci}", name=f"wk{ci}")
                    nc.sync.dma_start(
                        wkt[ci][:], wk_d[128 * ci:128 * (ci + 1), :])
                    wvt[ci] = wv_pool.tile([128, NKV * D], F16,
                                           tag=f"wv{ci}", name=f"wv{ci}")
                    nc.sync.dma_start(
                        wvt[ci][:], wv_d[128 * ci:128 * (ci + 1), :])

                NTH = 3          # x-column thirds
                QW = KVR // NTH  # 512 columns per third
                for qu in range(NTH):
                    qs = QW * qu
                    kps = [ps_kv.tile([128, QW], DT, tag=f"kps{m}",
                                      name=f"kps{m}")
                           for m in range(NKV // 2)]
                    vps = [ps_kv.tile([128, NKV * D], DT, tag=f"vps{st}",
                                      name=f"vps{st}")
                           for st in range(QW // 128)]
                    for ci in range(FD):
                        if qu == 2:
                            xt = x2t[ci]
                        else:
                            xt = xkv_pool.tile([128, QW], F16, tag="xkv",
                                               name="xkv")
                        nc.sync.dma_start(
                            xt[:], xkv_d[128 * ci:128 * (ci + 1), qs:qs + QW])
                        for m in range(NKV // 2):
                            nc.tensor.matmul(
                                kps[m][:], wkt[ci][:, 128 * m:128 * (m + 1)],
                                xt[:], start=(ci == 0), stop=(ci == FD - 1))
                        for st in range(QW // 128):
                            nc.tensor.matmul(
                                vps[st][:], xt[:, 128 * st:128 * (st + 1)],
                                wvt[ci][:], start=(ci == 0),
                                stop=(ci == FD - 1))
                    for m in range(NKV // 2):
                        _rope_write(nc, rtmp, kT[m][:, qs:qs + QW],
                                    kps[m][:], rkc[:, qs:qs + QW],
                                    rks[:, qs:qs + QW], QW)
                    for st in range(QW // 128):
                        tl = (QW // 128) * qu + st   # kv block 0..11
                        # v data for all 8 heads in one strided copy
                        nc.scalar.copy(
                            vext[:].rearrange("p (h b w) -> p h b w",
                                              h=NKV, b=NKB)[:, :, tl, 0:D],
                            vps[st][:].rearrange("p (h d) -> p h d", h=NKV))
                    # validity columns for this third's blocks, all heads
                    t0 = (QW // 128) * qu
                    nc.scalar.copy(
                        vext[:].rearrange("p (h b w) -> p h b w",
                                          h=NKV, b=NKB)[
                                              :, :, t0:t0 + QW // 128,
                                              D:D + 1],
                        kvv[:, t0:t0 + QW // 128].rearrange(
                            "p (o b) -> p o b", o=1).to_broadcast(
                                (128, NKV, QW // 128)))

            # ====== interleaved Q projection + attention ladder ======
            prefetch_wq(0)
            prefetch_wq(1)
            with tc.tile_pool(name="aT", bufs=1) as aT_pool:
                aT = [aT_pool.tile([128, CH], F16, tag=f"aT{i}",
                                   name=f"aT{i}") for i in range(NH // 2)]
                from contextlib import ExitStack
                att_stack = ExitStack()
                rtmpq = att_stack.enter_context(
                    tc.tile_pool(name="rope_tmp_q", bufs=3))
                pt_pool = att_stack.enter_context(
                    tc.tile_pool(name="pt", bufs=5))
                sm_pool = att_stack.enter_context(
                    tc.tile_pool(name="att_small", bufs=3))
                ps_att = att_stack.enter_context(
                    tc.tile_pool(name="ps_att", bufs=1, space="PSUM"))

                def q_quarter(sweep):
                    # projects heads 8*sweep .. 8*sweep+7 in two half-chains
                    # of 2 PSUM banks each
                    for half in range(2):
                        qps = [ps_q.tile([128, CH], DT, tag=f"qps{j}",
                                         name=f"qps{j}") for j in range(2)]
                        for ci in range(FD):
                            wt = wq_tiles[sweep][ci]
                            for j in range(2):
                                m4 = 2 * half + j
                                nc.tensor.matmul(
                                    qps[j][:],
                                    wt[:, 128 * m4:128 * (m4 + 1)],
                                    x2t[ci][:], start=(ci == 0),
                                    stop=(ci == FD - 1))
                        for j in range(2):
                            # m-tile (sweep, r): rows 0:64 = head 8*sweep+r
                            # (group 2*sweep, rep r), rows 64:128 = head
                            # 8*sweep+4+r (group 2*sweep+1, rep r) - both
                            # land in qT[sweep] column block r, partition-
                            # aligned (host packs Wq columns accordingly).
                            r = 2 * half + j
                            _rope_write(nc, rtmpq,
                                        qT[sweep][:, 512 * r:512 * (r + 1)],
                                        qps[j][:], rqc[:], rqs[:], CH)
                    prefetch_wq(sweep + 2)

                def attention_pair(gp):
                    kTt = kT[gp]
                    qTg = qT[gp]
                    for qt in range(NQT):
                        qv = [qTg[64 * h:64 * h + 64, :].rearrange(
                            "p (r t) -> p r t", r=REP)[
                                :, :, 128 * qt:128 * (qt + 1)]
                            for h in range(2)]
                        OT = [ps_att.tile([65, REP * 128], DT,
                                          tag=f"OT{h}", name=f"OT{h}",
                                          bufs=1) for h in range(2)]
                        for lk in range(NWB):
                            kb = qt + lk
                            ST = ps_att.tile([128, 2 * REP * 128], DT,
                                             tag="ST", name="ST", bufs=2)
                            for h in range(2):
                                nc.tensor.matmul(
                                    ST[:, 512 * h:512 * (h + 1)].rearrange(
                                        "p (r t) -> p r t", r=REP),
                                    kTt[64 * h:64 * h + 64,
                                        128 * kb:128 * (kb + 1)],
                                    qv[h], start=True, stop=True)
                            PT = pt_pool.tile([128, 1024], BF16,
                                              tag="PT", name="PT")
                            nc.scalar.activation(
                                PT[:], ST[:],
                                mybir.ActivationFunctionType.Exp)
                            if lk == 0:
                                nc.vector.tensor_mul(PT[:], PT[:],
                                                     mask_win[:])
                            elif lk == NWB - 1:
                                nc.vector.tensor_mul(PT[:], PT[:],
                                                     mask_causal[:])
                            for h in range(2):
                                g = 2 * gp + h
                                nc.tensor.matmul(
                                    OT[h][:],
                                    vext[:, VP * g + VW * kb:
                                         VP * g + VW * (kb + 1)],
                                    PT[:, 512 * h:512 * (h + 1)],
                                    start=(lk == 0), stop=(lk == NWB - 1))
                        for h in range(2):
                            g = 2 * gp + h
                            rcp = sm_pool.tile([1, REP * 128], DT,
                                               tag="rcp", name="rcp")
                            nc.vector.reciprocal(rcp[:], OT[h][64:65, :])
                            rcpb = sm_pool.tile([64, REP * 128], DT,
                                                tag="rcpb", name="rcpb")
                            nc.gpsimd.partition_broadcast(rcpb[:], rcp[:])
                            for r in range(REP):
                                hh = REP * g + r
                                nc.vector.tensor_mul(
                                    aT[hh // 2][64 * (hh % 2):
                                                64 * (hh % 2) + 64,
                                                128 * qt:128 * (qt + 1)],
                                    OT[h][0:64, 128 * r:128 * (r + 1)],
                                    rcpb[:, 128 * r:128 * (r + 1)])

                with tc.tile_pool(name="ps_q", bufs=1, space="PSUM") as ps_q:
                    for sweep in range(4):
                        if sweep >= 1:
                            attention_pair(sweep - 1)
                        q_quarter(sweep)
                attention_pair(3)
                att_stack.close()

                # ================= output projection =================
                with (
                    tc.tile_pool(name="wo_s", bufs=5) as wo_pool,
                    tc.tile_pool(name="ostage", bufs=4) as ostage,
                    tc.tile_pool(name="ps_o", bufs=1, space="PSUM") as ps_o,
                ):
                    for ocp in range(2):
                        ops = [ps_o.tile([128, 512], DT, tag=f"ops{j}",
                                         name=f"ops{j}")
                               for j in range(8)]
                        for k in range(FD):
                            wot = wo_pool.tile([128, 1024], F16, tag="wo",
                                               name="wo")
                            nc.sync.dma_start(
                                wot[:], wo_d[128 * k:128 * (k + 1),
                                             1024 * ocp:1024 * (ocp + 1)])
                            for o2 in range(2):
                                for tt in range(NQT):
                                    nc.tensor.matmul(
                                        ops[4 * o2 + tt][:],
                                        aT[k][:, 128 * tt:128 * (tt + 1)],
                                        wot[:, 512 * o2:512 * (o2 + 1)],
                                        start=(k == 0), stop=(k == FD - 1))
                        for o2 in range(2):
                            for tt in range(NQT):
                                oc = 2 * ocp + o2
                                st = ostage.tile([128, 512], DT, tag="stage",
                                                 name="stage")
                                nc.vector.tensor_copy(st[:],
                                                      ops[4 * o2 + tt][:])
                                nc.gpsimd.dma_start(
                                    out_d[128 * tt:128 * (tt + 1),
                                          512 * oc:512 * (oc + 1)], st[:])

    nc.compile()
    return nc


# old-dim -> new-dim pair interleave for one 64-dim head:
# new dim 2j holds old dim j, new dim 2j+1 holds old dim j+32.
_P64 = np.empty(64, np.int64)
_P64[0::2] = np.arange(32)
_P64[1::2] = np.arange(32, 64)


def _rope_tables(t_idx, scale):
    """cos/sin tables in pair-interleaved [d, t] layout, 2-head packed.

    Row 2j and 2j+1 carry cos(theta_j); sin row 2j is negated (rotate-half
    sign in the interleaved layout).  Rows 64:128 repeat for head 2."""
    inv_freq = 1.0 / (ROPE_BASE ** (np.arange(0, D, 2, dtype=np.float64) / D))
    ang = t_idx[None, :] * inv_freq[:, None]          # [32, n]
    cos1 = np.cos(ang)
    sin1 = np.sin(ang)
    n = ang.shape[1]
    cos64 = np.empty((64, n))
    cos64[0::2] = cos1
    cos64[1::2] = cos1
    sin64 = np.empty((64, n))
    sin64[0::2] = -sin1
    sin64[1::2] = sin1
    cos64 *= scale
    sin64 *= scale
    return (np.tile(cos64, (2, 1)).astype(np.float16),
            np.tile(sin64, (2, 1)).astype(np.float16))


def _permute_wk(Wk):
    """Pair-interleave each kv head's 64 dims in Wk's columns."""
    idx = np.concatenate([64 * h + _P64 for h in range(NKV)])
    return Wk[:, idx]


def _permute_wq(Wq):
    """Pack Wq columns so psum m-tile m = (sweep, r) holds head 8*sweep+r
    in rows 0:64 and head 8*sweep+4+r in rows 64:128, pair-interleaved."""
    cols = []
    for m in range(16):
        tau, r = divmod(m, 4)
        hA = 8 * tau + r
        hB = 8 * tau + 4 + r
        cols.append(64 * hA + _P64)
        cols.append(64 * hB + _P64)
    return Wq[:, np.concatenate(cols)]


def make_in_maps(x, Wq, Wk, Wv, Wo):
    x = np.asarray(x, np.float32)
    bf16 = ml_dtypes.bfloat16
    ins = []
    i = np.arange(128)
    masks = {
        "mask_win8": np.tile((i[:, None] > i[None, :]).astype(bf16),
                             (1, 2 * REP)),
        "mask_causal8": np.tile((i[:, None] <= i[None, :]).astype(bf16),
                                (1, 2 * REP)),
    }
    wq16 = np.ascontiguousarray(_permute_wq(np.asarray(Wq)), np.float16)
    wk16 = np.ascontiguousarray(_permute_wk(np.asarray(Wk)), np.float16)
    wv16 = np.ascontiguousarray(Wv, np.float16)
    wo16 = np.ascontiguousarray(Wo, np.float16)
    for c in range(NCORE):
        b, ch = divmod(c, 4)
        r0 = CH * ch
        kv0 = r0 - WIN
        xT = np.ascontiguousarray(x[b].T)             # [C, T]
        xkv = np.zeros((C, KVR), np.float16)
        pad = max(0, -kv0)
        xkv[:, pad:] = xT[:, kv0 + pad:r0 + CH].astype(np.float16)
        qc, qs = _rope_tables(np.arange(r0, r0 + CH, dtype=np.float64), SCALE)
        kc, ks = _rope_tables(np.arange(kv0, r0 + CH, dtype=np.float64), 1.0)
        kvvalid = np.zeros((128, NKB), bf16)
        for lk in range(NKB):
            kvvalid[:, lk] = (kv0 + 128 * lk + i >= 0).astype(bf16)
        ins.append({
            "xkv": xkv,
            "wq": wq16,
            "wk": wk16,
            "wv": wv16,
            "wo": wo16,
            "rope_q_cos": qc, "rope_q_sin": qs,
            "rope_k_cos": kc, "rope_k_sin": ks,
            "kvvalid": kvvalid,
            **masks,
        })
    return ins


_PROG_CACHE = {}


def get_program():
    if "nc" not in _PROG_CACHE:
        _PROG_CACHE["nc"] = build_program()
    return _PROG_CACHE["nc"]


def kernel(x, Wq, Wk, Wv, Wo):
    nc = get_program()
    ins = make_in_maps(x, Wq, Wk, Wv, Wo)
    res = run_bass_kernel_spmd(nc, ins, list(range(NCORE)))
    out = np.empty((B, T, C), np.float32)
    for c in range(NCORE):
        b, ch = divmod(c, 4)
        out[b, CH * ch:CH * (ch + 1), :] = res.results[c]["out"]
    return out


# revision 27
# speedup vs baseline: 1.0639x; 1.0157x over previous
"""Trainium2 Bass kernel for Mistral-style sliding-window GQA attention.

Problem: B=2, T=2048, C=2048, 32 q heads / 8 kv heads, head_dim=64,
sliding causal window 1024, RoPE, fp32.

Sharding (sequence-parallel, no cross-core communication):
  core c in 0..7 handles batch b=c//4 and contiguous 512-row chunk k=c%4.
  Each core computes q for its 512 rows, k/v for its rows plus a 1024-row
  halo (zero-padded before t=0), full attention for its rows over all 32
  heads, and the output projection for its rows.  Host gathers by
  concatenation only.

Device program (v2):
  - All DRAM inputs are fp16 (host-converted); matmul operands are fp16
    except the softmax path (PT / vext in bf16 for exp range safety);
    PSUM accumulation stays fp32.
  - All bulk weight/x DMAs ride the SP (sync) queue only, so HWDGE is
    uncontended and the ACT/DVE sequencers never stall behind DMA issue.
  - Q projection reuses the third-2 x tiles of the KV phase (the q token
    range is rows 1024:1536 of the kv range) - zero x re-loads.
  - RoPE: one ACT stage-copy (PSUM f32 -> SBUF fp16), then 6 DVE ops in
    fp16 (4 quarter muls for rotate-half * sin, full cos mul, full add).
  - Attention pairs the two kv groups of a ladder step into one
    [128, 1024] PSUM score tile so each Exp activation covers 2 groups.
  - scores are built in S^T = [key, query] layout with the 4 query heads
    packed side-by-side; PV uses V_ext as stationary so out^T lands in
    [d, t] with the softmax denominator in row 64 - no transposes.
  - no max-subtraction in softmax (scores/8 bounded ~15, exp safe in f32).
  - output projection accumulates in PSUM and DMAs PSUM->DRAM directly.
"""

import numpy as np
import ml_dtypes

import concourse.bass as bass
import concourse.mybir as mybir
import concourse.tile as tile
from concourse import bacc
from concourse.bass_utils import run_bass_kernel_spmd

B, T, C = 2, 2048, 2048
NH, NKV, D = 32, 8, 64
REP = NH // NKV
WIN = 1024
CH = 512          # q rows per core
KVR = CH + WIN    # kv rows per core (with halo)
NCORE = 8
DT = mybir.dt.float32
F16 = mybir.dt.float16
BF16 = mybir.dt.bfloat16
SCALE = 1.0 / np.sqrt(np.float32(D))
ROPE_BASE = 10000.0

FD = T // 128     # 16 contraction tiles of the model dim
NQT = CH // 128   # 4 q tiles per chunk
NKB = KVR // 128  # 12 kv blocks per core
NWB = 9           # kv blocks in the window of one q tile
VW = 65           # v_ext width per kv block (64 dims + validity column)
VP = NKB * VW     # per-head v_ext pitch (780)

# Head dims are PAIR-INTERLEAVED on host (new dim 2j = old j, 2j+1 = old
# j+32), so RoPE rotate-half is an adjacent-partition swap - expressible as
# a stream_shuffle (which permutes within 32-partition quadrants only).
_PAIR_SHUF = [i ^ 1 for i in range(32)]


def _rope_write(nc, pool, out_ap, ps, cosw, ssinw, n):
    """out = ps*cos + rot_half(ps)*sin on a [128, n] 2-head-packed tile.

    ps is a PSUM f32 tile; one ACT copy stages it to fp16 SBUF, then all
    multiplies/adds run on DVE in fp16.  ssinw rows carry the rotate-half
    signs (rows 0-31/64-95 negated) and any folded scale; cosw carries the
    same scale.  out_ap receives the fp16 result.
    """
    rs = pool.tile([128, n], F16, tag="rope_rs", name="rope_rs")
    nc.scalar.copy(rs[:], ps[:])
    # rotate-half = adjacent-partition swap in the pair-interleaved layout
    sw = pool.tile([128, n], F16, tag="rope_sw", name="rope_sw")
    nc.vector.stream_shuffle(sw[:], rs[:], _PAIR_SHUF)
    t2 = pool.tile([128, n], F16, tag="rope_t2", name="rope_t2")
    nc.vector.tensor_mul(t2[:], sw[:], ssinw[:])
    t1 = pool.tile([128, n], F16, tag="rope_t1", name="rope_t1")
    nc.vector.tensor_mul(t1[:], rs[:], cosw[:])
    nc.vector.tensor_add(out_ap, t1[:], t2[:])


def build_program():
    nc = bacc.Bacc("TRN2", target_bir_lowering=False, debug=False,
                   num_devices=NCORE)

    xkv_d = nc.dram_tensor("xkv", [C, KVR], F16, kind="ExternalInput")
    wq_d = nc.dram_tensor("wq", [C, NH * D], F16, kind="ExternalInput")
    wk_d = nc.dram_tensor("wk", [C, NKV * D], F16, kind="ExternalInput")
    wv_d = nc.dram_tensor("wv", [C, NKV * D], F16, kind="ExternalInput")
    wo_d = nc.dram_tensor("wo", [NH * D, C], F16, kind="ExternalInput")
    rqc_d = nc.dram_tensor("rope_q_cos", [128, CH], F16, kind="ExternalInput")
    rqs_d = nc.dram_tensor("rope_q_sin", [128, CH], F16, kind="ExternalInput")
    rkc_d = nc.dram_tensor("rope_k_cos", [128, KVR], F16, kind="ExternalInput")
    rks_d = nc.dram_tensor("rope_k_sin", [128, KVR], F16, kind="ExternalInput")
    kvv_d = nc.dram_tensor("kvvalid", [128, NKB], BF16, kind="ExternalInput")
    mw_d = nc.dram_tensor("mask_win8", [128, 1024], BF16, kind="ExternalInput")
    mc_d = nc.dram_tensor("mask_causal8", [128, 1024], BF16,
                          kind="ExternalInput")
    out_d = nc.dram_tensor("out", [CH, C], DT, kind="ExternalOutput")

    with tile.TileContext(nc) as tc:
        with (
            tc.tile_pool(name="const", bufs=1) as cpool,
            tc.tile_pool(name="qT", bufs=1) as qT_pool,
            tc.tile_pool(name="kT", bufs=1) as kT_pool,
            tc.tile_pool(name="vext", bufs=1) as v_pool,
            tc.tile_pool(name="x2", bufs=1) as x2_pool,
            tc.tile_pool(name="wq_s", bufs=1) as wq_pool,
        ):
            # ---- constants (small, persistent; SWDGE queue) ----
            mask_win = cpool.tile([128, 1024], BF16, tag="mw", name="mask_win")
            nc.gpsimd.dma_start(mask_win[:], mw_d[:, :])
            mask_causal = cpool.tile([128, 1024], BF16, tag="mc",
                                     name="mask_causal")
            nc.gpsimd.dma_start(mask_causal[:], mc_d[:, :])
            kvv = cpool.tile([128, NKB], BF16, tag="kvv", name="kvv")
            nc.gpsimd.dma_start(kvv[:], kvv_d[:, :])
            rkc = cpool.tile([128, KVR], F16, tag="rkc", name="rkc")
            nc.gpsimd.dma_start(rkc[:], rkc_d[:, :])
            rks = cpool.tile([128, KVR], F16, tag="rks", name="rks")
            nc.gpsimd.dma_start(rks[:], rks_d[:, :])
            rqc = cpool.tile([128, CH], F16, tag="rqc", name="rqc")
            nc.gpsimd.dma_start(rqc[:], rqc_d[:, :])
            rqs = cpool.tile([128, CH], F16, tag="rqs", name="rqs")
            nc.gpsimd.dma_start(rqs[:], rqs_d[:, :])

            # qT: [d, t] grouped by kv head.  Tile j rows 0:64 = group 2j
            # (its 4 heads side by side, 512 cols each), rows 64:128 =
            # group 2j+1, so QK lhsT and rhs share a base partition.
            qT = [qT_pool.tile([128, REP * CH], F16, tag=f"qT{i}",
                               name=f"qT{i}") for i in range(NKV // 2)]
            # kT: [d, t] packed 2 kv heads per tile.
            kT = [kT_pool.tile([128, KVR], F16, tag=f"kT{i}", name=f"kT{i}")
                  for i in range(NKV // 2)]
            # vext: one tile, head kvh at pitch VP; per block 64 dims+validity
            vext = v_pool.tile([128, NKV * VP], BF16, tag="vext", name="vext")
            # third-2 x tiles stay resident: they double as the q-range x.
            x2t = [x2_pool.tile([128, CH], F16, tag=f"x2_{ci}",
                                name=f"x2_{ci}") for ci in range(FD)]
            # wq tiles: rotating pool, prefetched 2 sweeps ahead (SP queue)
            wq_tiles = {}

            def prefetch_wq(s):
                if s > 3 or s in wq_tiles:
                    return
                tl = []
                for ci in range(FD):
                    t = wq_pool.tile([128, 512], F16, tag="wq", name="wq",
                                     bufs=32)
                    nc.sync.dma_start(
                        t[:], wq_d[128 * ci:128 * (ci + 1),
                                   512 * s:512 * (s + 1)])
                    tl.append(t)
                wq_tiles[s] = tl

            # ================= KV projection =================
            with (
                tc.tile_pool(name="wk_res", bufs=1) as wk_pool,
                tc.tile_pool(name="wv_res", bufs=1) as wv_pool,
                tc.tile_pool(name="xkv_s", bufs=6) as xkv_pool,
                tc.tile_pool(name="rope_tmp", bufs=3) as rtmp,
                tc.tile_pool(name="ps_kv", bufs=1, space="PSUM") as ps_kv,
            ):
                # interleave weight and first-third x DMA issue so the first
                # matmul's inputs arrive within a couple of microseconds
                wkt = {}
                wvt = {}
                x0t = {}
                for ci in range(FD):
                    wkt[ci] = wk_pool.tile([128, NKV * D], F16,
                                           tag=f"wk{ci}", name=f"wk{ci}")
                    nc.sync.dma_start(
                        wkt[ci][:], wk_d[128 * ci:128 * (ci + 1), :])
                    x0t[ci] = xkv_pool.tile([128, KVR // 3], F16, tag="xkv",
                                            name="xkv")
                    nc.sync.dma_start(
                        x0t[ci][:], xkv_d[128 * ci:128 * (ci + 1),
                                          0:KVR // 3])
                    wvt[ci] = wv_pool.tile([128, NKV * D], F16,
                                           tag=f"wv{ci}", name=f"wv{ci}")
                    nc.sync.dma_start(
                        wvt[ci][:], wv_d[128 * ci:128 * (ci + 1), :])

                NTH = 3          # x-column thirds
                QW = KVR // NTH  # 512 columns per third
                for qu in range(NTH):
                    qs = QW * qu
                    kps = [ps_kv.tile([128, QW], DT, tag=f"kps{m}",
                                      name=f"kps{m}")
                           for m in range(NKV // 2)]
                    vps = [ps_kv.tile([128, NKV * D], DT, tag=f"vps{st}",
                                      name=f"vps{st}")
                           for st in range(QW // 128)]
                    for ci in range(FD):
                        if qu == 0:
                            xt = x0t[ci]
                        elif qu == 2:
                            xt = x2t[ci]
                            nc.sync.dma_start(
                                xt[:], xkv_d[128 * ci:128 * (ci + 1),
                                             qs:qs + QW])
                        else:
                            xt = xkv_pool.tile([128, QW], F16, tag="xkv",
                                               name="xkv")
                            nc.sync.dma_start(
                                xt[:], xkv_d[128 * ci:128 * (ci + 1),
                                             qs:qs + QW])
                        for m in range(NKV // 2):
                            nc.tensor.matmul(
                                kps[m][:], wkt[ci][:, 128 * m:128 * (m + 1)],
                                xt[:], start=(ci == 0), stop=(ci == FD - 1))
                        for st in range(QW // 128):
                            nc.tensor.matmul(
                                vps[st][:], xt[:, 128 * st:128 * (st + 1)],
                                wvt[ci][:], start=(ci == 0),
                                stop=(ci == FD - 1))
                    for m in range(NKV // 2):
                        _rope_write(nc, rtmp, kT[m][:, qs:qs + QW],
                                    kps[m][:], rkc[:, qs:qs + QW],
                                    rks[:, qs:qs + QW], QW)
                    for st in range(QW // 128):
                        tl = (QW // 128) * qu + st   # kv block 0..11
                        # v data for all 8 heads in one strided copy
                        dst = vext[:].rearrange("p (h b w) -> p h b w",
                                                h=NKV, b=NKB)[:, :, tl, 0:D]
                        src = vps[st][:].rearrange("p (h d) -> p h d", h=NKV)
                        if qu == 2:
                            nc.vector.tensor_copy(dst, src)
                        else:
                            nc.scalar.copy(dst, src)
                    # validity columns for this third's blocks, all heads
                    t0 = (QW // 128) * qu
                    nc.scalar.copy(
                        vext[:].rearrange("p (h b w) -> p h b w",
                                          h=NKV, b=NKB)[
                                              :, :, t0:t0 + QW // 128,
                                              D:D + 1],
                        kvv[:, t0:t0 + QW // 128].rearrange(
                            "p (o b) -> p o b", o=1).to_broadcast(
                                (128, NKV, QW // 128)))

            # ====== interleaved Q projection + attention ladder ======
            prefetch_wq(0)
            prefetch_wq(1)
            with tc.tile_pool(name="aT", bufs=1) as aT_pool:
                aT = [aT_pool.tile([128, CH], F16, tag=f"aT{i}",
                                   name=f"aT{i}") for i in range(NH // 2)]
                from contextlib import ExitStack
                att_stack = ExitStack()
                rtmpq = att_stack.enter_context(
                    tc.tile_pool(name="rope_tmp_q", bufs=3))
                pt_pool = att_stack.enter_context(
                    tc.tile_pool(name="pt", bufs=5))
                sm_pool = att_stack.enter_context(
                    tc.tile_pool(name="att_small", bufs=3))
                ps_att_stack = ExitStack()
                ps_att = ps_att_stack.enter_context(
                    tc.tile_pool(name="ps_att", bufs=1, space="PSUM"))

                def q_quarter(sweep):
                    # projects heads 8*sweep .. 8*sweep+7 in two half-chains
                    # of 2 PSUM banks each
                    for half in range(2):
                        qps = [ps_q.tile([128, CH], DT, tag=f"qps{j}",
                                         name=f"qps{j}") for j in range(2)]
                        for ci in range(FD):
                            wt = wq_tiles[sweep][ci]
                            for j in range(2):
                                m4 = 2 * half + j
                                nc.tensor.matmul(
                                    qps[j][:],
                                    wt[:, 128 * m4:128 * (m4 + 1)],
                                    x2t[ci][:], start=(ci == 0),
                                    stop=(ci == FD - 1))
                        for j in range(2):
                            # m-tile (sweep, r): rows 0:64 = head 8*sweep+r
                            # (group 2*sweep, rep r), rows 64:128 = head
                            # 8*sweep+4+r (group 2*sweep+1, rep r) - both
                            # land in qT[sweep] column block r, partition-
                            # aligned (host packs Wq columns accordingly).
                            r = 2 * half + j
                            _rope_write(nc, rtmpq,
                                        qT[sweep][:, 512 * r:512 * (r + 1)],
                                        qps[j][:], rqc[:], rqs[:], CH)
                    prefetch_wq(sweep + 2)

                def attention_pair(gp, ps_pool, st_bufs, fill=None):
                    kTt = kT[gp]
                    qTg = qT[gp]
                    for qt in range(NQT):
                        qv = [qTg[64 * h:64 * h + 64, :].rearrange(
                            "p (r t) -> p r t", r=REP)[
                                :, :, 128 * qt:128 * (qt + 1)]
                            for h in range(2)]
                        OT = [ps_pool.tile([65, REP * 128], DT,
                                           tag=f"OT{h}", name=f"OT{h}",
                                           bufs=1) for h in range(2)]
                        for lk in range(NWB):
                            kb = qt + lk
                            ST = ps_pool.tile([128, 2 * REP * 128], DT,
                                              tag="ST", name="ST",
                                              bufs=st_bufs)
                            for h in range(2):
                                nc.tensor.matmul(
                                    ST[:, 512 * h:512 * (h + 1)].rearrange(
                                        "p (r t) -> p r t", r=REP),
                                    kTt[64 * h:64 * h + 64,
                                        128 * kb:128 * (kb + 1)],
                                    qv[h], start=True, stop=True)
                            PT = pt_pool.tile([128, 1024], BF16,
                                              tag="PT", name="PT")
                            nc.scalar.activation(
                                PT[:], ST[:],
                                mybir.ActivationFunctionType.Exp)
                            if lk == 0:
                                nc.vector.tensor_mul(PT[:], PT[:],
                                                     mask_win[:])
                            elif lk == NWB - 1:
                                nc.vector.tensor_mul(PT[:], PT[:],
                                                     mask_causal[:])
                            for h in range(2):
                                g = 2 * gp + h
                                nc.tensor.matmul(
                                    OT[h][:],
                                    vext[:, VP * g + VW * kb:
                                         VP * g + VW * (kb + 1)],
                                    PT[:, 512 * h:512 * (h + 1)],
                                    start=(lk == 0), stop=(lk == NWB - 1))
                        for h in range(2):
                            g = 2 * gp + h
                            rcp = sm_pool.tile([1, REP * 128], DT,
                                               tag="rcp", name="rcp")
                            nc.vector.reciprocal(rcp[:], OT[h][64:65, :])
                            rcpb = sm_pool.tile([64, REP * 128], DT,
                                                tag="rcpb", name="rcpb")
                            nc.gpsimd.partition_broadcast(rcpb[:], rcp[:])
                            for r in range(REP):
                                hh = REP * g + r
                                nc.vector.tensor_mul(
                                    aT[hh // 2][64 * (hh % 2):
                                                64 * (hh % 2) + 64,
                                                128 * qt:128 * (qt + 1)],
                                    OT[h][0:64, 128 * r:128 * (r + 1)],
                                    rcpb[:, 128 * r:128 * (r + 1)])
                        if fill is not None:
                            fill(qt)

                with tc.tile_pool(name="ps_q", bufs=1, space="PSUM") as ps_q:
                    for sweep in range(4):
                        if sweep >= 1:
                            attention_pair(sweep - 1, ps_att, 2)
                        q_quarter(sweep)
                ps_att_stack.close()

                # ======= output projection (round 0 under attn3) =======
                with (
                    tc.tile_pool(name="wo_s", bufs=6) as wo_pool,
                    tc.tile_pool(name="ostage", bufs=4) as ostage,
                    tc.tile_pool(name="ps_o", bufs=1, space="PSUM") as ps_o,
                ):
                    def load_wo(k, oc):
                        t = wo_pool.tile([128, 512], F16, tag="wo",
                                         name="wo", bufs=6)
                        nc.sync.dma_start(
                            t[:], wo_d[128 * k:128 * (k + 1),
                                       512 * oc:512 * (oc + 1)])
                        return t

                    def oproj_mms(ops, oc, k):
                        wot = load_wo(k, oc)
                        for tt in range(NQT):
                            nc.tensor.matmul(
                                ops[tt][:],
                                aT[k][:, 128 * tt:128 * (tt + 1)],
                                wot[:], start=(k == 0), stop=(k == FD - 1))

                    def flush_ops(ops, oc):
                        for tt in range(NQT):
                            st = ostage.tile([128, 512], DT,
                                             tag=f"stage{tt % 2}",
                                             name="stage", bufs=2)
                            if tt % 2 == 0:
                                nc.vector.tensor_copy(st[:], ops[tt][:])
                            else:
                                nc.scalar.copy(st[:], ops[tt][:])
                            eng = nc.gpsimd if tt % 2 == 0 else nc.scalar
                            eng.dma_start(
                                out_d[128 * tt:128 * (tt + 1),
                                      512 * oc:512 * (oc + 1)], st[:])

                    ops0 = [ps_o.tile([128, 512], DT, tag=f"o0_{tt}",
                                      name=f"o0_{tt}")
                            for tt in range(NQT)]

                    def fill0(qt):
                        # 12 dense o-proj matmuls per attn3 unit (k 0..11
                        # use aT of heads 0..23, all ready before attn3)
                        for k in range(3 * qt, 3 * qt + 3):
                            oproj_mms(ops0, 0, k)

                    with tc.tile_pool(name="ps_att3", bufs=1,
                                      space="PSUM") as ps_att3:
                        attention_pair(3, ps_att3, 1, fill=fill0)
                    for k in range(12, FD):
                        oproj_mms(ops0, 0, k)
                    flush_ops(ops0, 0)
                    with tc.tile_pool(name="ps_o2", bufs=1,
                                      space="PSUM") as ps_o2:
                        for oc in range(1, 4):
                            pool = ps_o2 if oc % 2 else ps_o
                            ops = [pool.tile([128, 512], DT,
                                             tag=f"o{oc % 2}_{tt}",
                                             name=f"o{oc % 2}_{tt}")
                                   for tt in range(NQT)]
                            for k in range(FD):
                                oproj_mms(ops, oc, k)
                            flush_ops(ops, oc)
                att_stack.close()

    nc.compile()
    return nc


# old-dim -> new-dim pair interleave for one 64-dim head:
# new dim 2j holds old dim j, new dim 2j+1 holds old dim j+32.
_P64 = np.empty(64, np.int64)
_P64[0::2] = np.arange(32)
_P64[1::2] = np.arange(32, 64)


def _rope_tables(t_idx, scale):
    """cos/sin tables in pair-interleaved [d, t] layout, 2-head packed.

    Row 2j and 2j+1 carry cos(theta_j); sin row 2j is negated (rotate-half
    sign in the interleaved layout).  Rows 64:128 repeat for head 2."""
    inv_freq = 1.0 / (ROPE_BASE ** (np.arange(0, D, 2, dtype=np.float64) / D))
    ang = t_idx[None, :] * inv_freq[:, None]          # [32, n]
    cos1 = np.cos(ang)
    sin1 = np.sin(ang)
    n = ang.shape[1]
    cos64 = np.empty((64, n))
    cos64[0::2] = cos1
    cos64[1::2] = cos1
    sin64 = np.empty((64, n))
    sin64[0::2] = -sin1
    sin64[1::2] = sin1
    cos64 *= scale
    sin64 *= scale
    return (np.tile(cos64, (2, 1)).astype(np.float16),
            np.tile(sin64, (2, 1)).astype(np.float16))


def _permute_wk(Wk):
    """Pair-interleave each kv head's 64 dims in Wk's columns."""
    idx = np.concatenate([64 * h + _P64 for h in range(NKV)])
    return Wk[:, idx]


def _permute_wq(Wq):
    """Pack Wq columns so psum m-tile m = (sweep, r) holds head 8*sweep+r
    in rows 0:64 and head 8*sweep+4+r in rows 64:128, pair-interleaved."""
    cols = []
    for m in range(16):
        tau, r = divmod(m, 4)
        hA = 8 * tau + r
        hB = 8 * tau + 4 + r
        cols.append(64 * hA + _P64)
        cols.append(64 * hB + _P64)
    return Wq[:, np.concatenate(cols)]


def make_in_maps(x, Wq, Wk, Wv, Wo):
    x = np.asarray(x, np.float32)
    bf16 = ml_dtypes.bfloat16
    ins = []
    i = np.arange(128)
    masks = {
        "mask_win8": np.tile((i[:, None] > i[None, :]).astype(bf16),
                             (1, 2 * REP)),
        "mask_causal8": np.tile((i[:, None] <= i[None, :]).astype(bf16),
                                (1, 2 * REP)),
    }
    wq16 = np.ascontiguousarray(_permute_wq(np.asarray(Wq)), np.float16)
    wk16 = np.ascontiguousarray(_permute_wk(np.asarray(Wk)), np.float16)
    wv16 = np.ascontiguousarray(Wv, np.float16)
    wo16 = np.ascontiguousarray(Wo, np.float16)
    for c in range(NCORE):
        b, ch = divmod(c, 4)
        r0 = CH * ch
        kv0 = r0 - WIN
        xT = np.ascontiguousarray(x[b].T)             # [C, T]
        xkv = np.zeros((C, KVR), np.float16)
        pad = max(0, -kv0)
        xkv[:, pad:] = xT[:, kv0 + pad:r0 + CH].astype(np.float16)
        qc, qs = _rope_tables(np.arange(r0, r0 + CH, dtype=np.float64), SCALE)
        kc, ks = _rope_tables(np.arange(kv0, r0 + CH, dtype=np.float64), 1.0)
        kvvalid = np.zeros((128, NKB), bf16)
        for lk in range(NKB):
            kvvalid[:, lk] = (kv0 + 128 * lk + i >= 0).astype(bf16)
        ins.append({
            "xkv": xkv,
            "wq": wq16,
            "wk": wk16,
            "wv": wv16,
            "wo": wo16,
            "rope_q_cos": qc, "rope_q_sin": qs,
            "rope_k_cos": kc, "rope_k_sin": ks,
            "kvvalid": kvvalid,
            **masks,
        })
    return ins


_PROG_CACHE = {}


def get_program():
    if "nc" not in _PROG_CACHE:
        _PROG_CACHE["nc"] = build_program()
    return _PROG_CACHE["nc"]


def kernel(x, Wq, Wk, Wv, Wo):
    nc = get_program()
    ins = make_in_maps(x, Wq, Wk, Wv, Wo)
    res = run_bass_kernel_spmd(nc, ins, list(range(NCORE)))
    out = np.empty((B, T, C), np.float32)
    for c in range(NCORE):
        b, ch = divmod(c, 4)
        out[b, CH * ch:CH * (ch + 1), :] = res.results[c]["out"]
    return out
